# revision 1
# baseline (speedup 1.0000x reference)
"""Trainium2 Bass kernel for nn_BertClassifier_37907381354985.

Span-pair classifier: for every valid span (i, j) with i <= j < i + 30 over
L=128 tokens, compute log_softmax(relu(x_i W1a + x_j W1b + ind*w1c + b1) W2 + b2).

Strategy (data-parallel over batch, 2 batches per core on 8 cores):
  * Algebraic restructuring: precompute AT = W1a^T X^T and CT = W1b^T X^T
    ([H1, L] per batch) on the tensor engine, then every span's hidden vector
    is AT[:, i] + CT[:, j] -- for spans grouped by width w = j - i this is a
    *shifted add* along the free axis (no gather).  ~27x fewer matmul FLOPs
    than the reference formulation.
  * The pred-span indicator decomposes as
        ind = 1{i>=s} - 1{j>e} + 1{i<s & j>e} + 1{i==s & j==e}
    The first two terms are rank-1 in (h x token) and are folded into the
    AT/CT matmuls via augmented contraction rows (u[i]=1{i>=s}, ones,
    v[j]=1{j>e} appended to X^T; w1c/b1/-w1c rows appended to the weights).
    The remaining sparse correction (contained-span + exact-span slots) is a
    host-computed per-span row, applied on-device as one fused
    (q * w1c[p]) + h pass.  The program therefore depends only on shapes,
    never on input values.
  * h is assembled in bf16 [H1-tile, 30*128] diagonal-major layout (each
    diagonal padded to 128 slots; pad slots compute garbage that is never
    read back).  relu via tensor_scalar_max.  Second matmul streams
    W2 (+b2 via an appended ones-row in h) into [128-span, 40] PSUM tiles.
    log_softmax over the free axis in fp32 (exp on ScalarE, sum/sub on DVE;
    max-subtraction skipped -- logits are O(10) so exp cannot overflow).
  * Host side: shard batches, pre-transpose/cast inputs, and invert the
    diagonal-major ordering back to the reference's row-major span order.
"""

import numpy as np

L = 128
D = 768
H1 = 770
OUT = 40
WMAX = 30
B = 16
NCORES = 8
BL = B // NCORES          # batches per core
HT = 110                  # h rows per k-tile
NK = 7                    # h k-tiles (7 * 110 = 770)
ND = 7                    # contraction tiles (6 * 128 data + 1 aug tile)
DAUG = ND * 128           # padded contraction rows
FDH = WMAX * L            # diagonal-major span slots per batch (3840)
NCH = FDH // L            # span chunks of 128 (= WMAX)

_prog_cache = {}


def _f32(x):
    return np.ascontiguousarray(np.asarray(x, dtype=np.float32))


def _bf16(x):
    import ml_dtypes
    return np.ascontiguousarray(np.asarray(x, dtype=np.float32).astype(ml_dtypes.bfloat16))


def _view(base, col_off, dims):
    """Free-axis re-view of a 2D [P, F] SBUF access pattern.

    dims: list of (step, count) free dims, outer->inner.  Partition dim kept.
    """
    from concourse.ap import AP
    ap0 = list(base.ap)
    part = [list(ap0[0])]
    return AP(
        tensor=base.tensor,
        offset=base.offset + col_off,
        ap=part + [[int(s), int(c)] for s, c in dims],
    )


def _make_tc_class():
    import concourse.mybir as mybir
    from concourse.tile import TileContext
    from concourse.vector_clock import ScopedClock

    # --- TileContext variant for this container's walrus build, which encodes
    # at most ONE sync-wait condition per instruction.  Tile freely attaches
    # several waits to one instruction, so (a) every scheduled instruction
    # with more than one wait gets the excess hoisted onto same-engine NOPs
    # inserted directly before it, and (b) the kernel-tail drain (one wait per
    # logical processor) is split the same way.  Waits are AND conditions, so
    # any same-engine placement before the original instruction preserves the
    # happens-before edges.
    class SplitDrainTileContext(TileContext):
        def _split_multi_waits(self, ordered):
            for bb_name, insts in ordered.items():
                out_list = []
                for inst in insts:
                    si = getattr(inst, "sync_info", None)
                    waits = list(si.on_wait) if si is not None and si.on_wait else []
                    if len(waits) > 1:
                        for w in waits[:-1]:
                            nop = mybir.InstNoOp(
                                name=self.nc.get_next_instruction_name(),
                                engine=inst.engine,
                                sync_info=mybir.SyncInfo(on_wait=[w], on_update=[]),
                                text_hint="waitsplit",
                                bass_nofuse=True,
                            )
                            self.nc.register_instruction(nop, overwrite=True)
                            out_list.append(nop)
                        inst.sync_info = mybir.SyncInfo(
                            on_wait=[waits[-1]],
                            on_update=list(si.on_update or []),
                        )
                    out_list.append(inst)
                insts[:] = out_list

        def _lower_ordered_insts(self, ordered):
            self._split_multi_waits(ordered)
            super()._lower_ordered_insts(ordered)

        def _drain_and_barrier(self, tick_clock, wait_clock):
            drain_inst = self.nc.sync.drain()
            wait_clock.add_sem_waits(
                drain_inst.ins, ScopedClock({None: tick_clock.global_clock})
            )
            si = drain_inst.ins.sync_info
            waits = list(si.on_wait) if si is not None and si.on_wait else []
            if len(waits) > 1:
                drain_inst.ins.sync_info = mybir.SyncInfo(
                    on_wait=waits[:1], on_update=list(si.on_update or [])
                )
                for i in range(1, len(waits)):
                    nop = self.nc.sync.nop(nofuse=True, hint="drain_split")
                    nop.ins.sync_info = mybir.SyncInfo(
                        on_wait=waits[i : i + 1], on_update=[]
                    )
            self.nc.all_engine_barrier()
            assert self.sems is not None
            popped = self.nc._tile_sem_poison_stack.pop()
            assert popped is self._sem_poison
            self.nc.clear_and_free_semaphores(list(self.sems.allocated().values()))
            self.nc.all_engine_barrier()

    return SplitDrainTileContext


def _build_program(cfg=None):
    """cfg: dict with per-(b,k) engine choices:
    - 'tt_even', 'tt_odd': set of (b,k) indices whose assembly add runs on
      Pool instead of DVE
    - 'relu': dict (b,k) -> 'act' | 'dve' | 'pool'
    """
    if cfg is None:
        cfg = {}
    tt_even_pool = set(cfg.get("tt_even", ()))
    tt_odd_pool = set(cfg.get("tt_odd", ()))
    relu_eng = cfg.get("relu", {})
    emission = cfg.get("emission", "stages")   # "stages" | "tiles"
    tile_order = cfg.get("tile_order", "bk")   # "bk" | "kb"
    qw_bufs = cfg.get("qw_bufs", 2)
    loop_reps = cfg.get("loop_reps", 0)        # >0: wrap compute in For_i (timing)
    assembly = cfg.get("assembly", "split")    # "split" (even/odd 2x-aligned) | "single"
    import concourse.bass as bass
    import concourse.mybir as mybir
    from concourse.tile import TileContext
    from concourse.vector_clock import ScopedClock

    SplitDrainTileContext = _make_tc_class()

    dt = mybir.dt
    Alu = mybir.AluOpType
    Act = mybir.ActivationFunctionType

    nc = bass.Bass("TRN2", target_bir_lowering=False, debug=False)

    vp = nc.dram_tensor("vp", [ND, 128, 128 * BL], dt.bfloat16, kind="ExternalInput")
    wa = nc.dram_tensor("wa", [ND, 128, H1], dt.bfloat16, kind="ExternalInput")
    wc = nc.dram_tensor("wc", [ND, 128, H1], dt.bfloat16, kind="ExternalInput")
    w2c = nc.dram_tensor("w2c", [H1 + 1, OUT], dt.bfloat16, kind="ExternalInput")
    w1cc = nc.dram_tensor("w1cc", [H1, 1], dt.float32, kind="ExternalInput")
    qr = nc.dram_tensor("qr", [BL, FDH], dt.bfloat16, kind="ExternalInput")
    ones_d = nc.dram_tensor("ones_d", [1, FDH], dt.bfloat16, kind="ExternalInput")
    # [b, span-in-chunk, chunk*OUT+class]: keeps the store one large
    # contiguous-per-partition DMA per batch (128 x 4.8KB descriptors).
    out = nc.dram_tensor("out", [BL, L, NCH * OUT], dt.float32, kind="ExternalOutput")

    with SplitDrainTileContext(nc) as tc:
        import contextlib
        with contextlib.ExitStack() as ctx:
            const = ctx.enter_context(tc.tile_pool(name="const", bufs=1))
            combp = ctx.enter_context(tc.tile_pool(name="comb", bufs=1))
            hp = ctx.enter_context(tc.tile_pool(name="h", bufs=1))
            acp = ctx.enter_context(tc.tile_pool(name="acpsum", bufs=2, space="PSUM"))
            w2p = ctx.enter_context(tc.tile_pool(name="w2psum", bufs=1, space="PSUM"))
            smp = ctx.enter_context(tc.tile_pool(name="smx", bufs=1))
            qwp = ctx.enter_context(tc.tile_pool(name="qw", bufs=qw_bufs))

            # ---- constant loads -------------------------------------------------
            vt, wat, wct, w2t, w1cs = [], [], [], [], []
            for d in range(ND):
                t = const.tile([128, 128 * BL], dt.bfloat16, tag=f"vt{d}")
                nc.sync.dma_start(out=t[:], in_=vp[d])
                vt.append(t)
                # split weight loads into column chunks: the A/C matmul for
                # h-tile k only reads cols [110k, 110k+110), and Tile tracks
                # subtile deps, so early h-tiles start ~3x sooner
                t = const.tile([128, H1], dt.bfloat16, tag=f"wat{d}")
                for c0, c1 in ((0, 220), (220, 440), (440, 660), (660, H1)):
                    nc.sync.dma_start(out=t[:, c0:c1], in_=wa[d][:, c0:c1])
                wat.append(t)
                t = const.tile([128, H1], dt.bfloat16, tag=f"wct{d}")
                for c0, c1 in ((0, 220), (220, 440), (440, 660), (660, H1)):
                    nc.sync.dma_start(out=t[:, c0:c1], in_=wc[d][:, c0:c1])
                wct.append(t)
            for k in range(NK):
                kk = HT + 1 if k == NK - 1 else HT
                t = const.tile([kk, OUT], dt.bfloat16, tag=f"w2t{k}")
                nc.sync.dma_start(out=t[:], in_=w2c[HT * k : HT * k + kk, :])
                w2t.append(t)
                t = const.tile([HT, 1], dt.float32, tag=f"w1cs{k}")
                nc.sync.dma_start(out=t[:], in_=w1cc[HT * k : HT * k + HT, :])
                w1cs.append(t)
            qb = []
            for b in range(BL):
                t = const.tile([HT, FDH], dt.bfloat16, tag=f"qb{b}")
                nc.sync.dma_start(out=t[:], in_=qr[b : b + 1, :].partition_broadcast(HT))
                qb.append(t)
            zt = const.tile([HT, L], dt.bfloat16, tag="zeros")
            nc.gpsimd.memset(zt[:], 0.0)

            if loop_reps:
                # timing mode: repeat the whole compute pipeline on-device so
                # per-iteration time can be extracted from two rep counts
                ctx.enter_context(tc.For_i(0, loop_reps, 1))

            # ---- phase 1: AT'/CT_hi matmuls + bf16 copies ----------------------
            comb, sh = [], []
            for k in range(NK):
                ps = acp.tile([HT, 512], dt.float32, tag="acps")
                for d in range(ND):
                    nc.tensor.matmul(
                        ps[:, 0 : 128 * BL],
                        lhsT=wat[d][:, HT * k : HT * k + HT],
                        rhs=vt[d][:],
                        start=(d == 0),
                        stop=(d == ND - 1),
                    )
                for d in range(ND):
                    nc.tensor.matmul(
                        ps[:, 128 * BL : 256 * BL],
                        lhsT=wct[d][:, HT * k : HT * k + HT],
                        rhs=vt[d][:],
                        start=(d == 0),
                        stop=(d == ND - 1),
                    )
                cb = combp.tile([HT, 544], dt.bfloat16, tag=f"comb{k}")
                nc.scalar.copy(cb[:, 0:512], ps[:, 0:512])
                comb.append(cb)
                shk = []
                for b in range(BL):
                    s = combp.tile([HT, 158], dt.bfloat16, tag=f"sh{k}_{b}")
                    nc.vector.tensor_copy(
                        s[:], _view(cb[:, :], 257 + 128 * b, [(1, 158)])
                    )
                    shk.append(s)
                sh.append(shk)

            # ---- phase 2: assembly + correction + relu -------------------------
            # Stage-ordered emission: engines have short in-order queues, so
            # interleaving a tile's cross-engine chain head-of-line-blocks the
            # fast engine behind the slow one.  Emit per-stage loops instead.
            NW2 = WMAX // 2
            QI = 98
            ht = [[None] * NK for _ in range(BL)]
            if tile_order == "kb":
                tiles = [(b, k) for k in range(NK) for b in range(BL)]
            else:
                tiles = [(b, k) for b in range(BL) for k in range(NK)]
            for b, k in tiles:
                parts = HT + 1 if k == NK - 1 else HT
                h = hp.tile([parts, FDH], dt.bfloat16, tag=f"h{b}_{k}")
                ht[b][k] = h
                if k == NK - 1:
                    # ones row for the b2 ktile at partition 110 (DMA writes
                    # have no partition-alignment restriction)
                    nc.sync.dma_start(out=h[HT : HT + 1, :], in_=ones_d.ap())

            def emit_odd(b, k):
                od_eng = nc.gpsimd if (b, k) in tt_odd_pool else nc.vector
                if assembly == "single":
                    # all 30 diagonals in one op: in1 steps (1, 30)(1, 128)
                    od_eng.tensor_tensor(
                        out=_view(ht[b][k][0:HT, :], 0, [(L, WMAX), (1, L)]),
                        in0=_view(comb[k][:, :], 128 * b, [(0, WMAX), (1, L)]),
                        in1=_view(comb[k][:, :], 128 * BL + 128 * b, [(1, WMAX), (1, L)]),
                        op=Alu.add,
                    )
                    return
                # odd diagonals w = 2*w2+1: CT_hi[i + w] = sh[2*w2 + i]
                od_eng.tensor_tensor(
                    out=_view(ht[b][k][0:HT, :], 128, [(256, NW2), (1, L)]),
                    in0=_view(comb[k][:, :], 128 * b, [(0, NW2), (1, L)]),
                    in1=_view(sh[k][b][:, :], 0, [(2, NW2), (1, L)]),
                    op=Alu.add,
                )

            def emit_even(b, k):
                if assembly == "single":
                    return
                # even diagonals w = 2*w2: CT_hi[i + w] straight from comb
                ev_eng = nc.gpsimd if (b, k) in tt_even_pool else nc.vector
                ev_eng.tensor_tensor(
                    out=_view(ht[b][k][0:HT, :], 0, [(256, NW2), (1, L)]),
                    in0=_view(comb[k][:, :], 128 * b, [(0, NW2), (1, L)]),
                    in1=_view(comb[k][:, :], 128 * BL + 128 * b, [(2, NW2), (1, L)]),
                    op=Alu.add,
                )

            def emit_qmul(b, k):
                # sparse indicator correction: h += q * w1c[p].  q can only be
                # nonzero at i < 98 of each diagonal (contained spans need
                # i < s <= 97, the exact span sits at i = s), so restrict the
                # pass there.  Split into tensor_scalar mult + tensor_tensor
                # add -- scalar_tensor_tensor only runs at 1x.
                qw = qwp.tile([HT, WMAX * QI], dt.bfloat16, tag="qw")
                nc.vector.tensor_scalar_mul(
                    qw[:], _view(qb[b][:, :], 0, [(L, WMAX), (1, QI)]),
                    w1cs[k][:, 0:1],
                )
                return qw

            def emit_qadd(b, k, qw):
                nc.vector.tensor_tensor(
                    out=_view(ht[b][k][0:HT, :], 0, [(L, WMAX), (1, QI)]),
                    in0=_view(ht[b][k][0:HT, :], 0, [(L, WMAX), (1, QI)]),
                    in1=qw[:].rearrange("p (w i) -> p w i", i=QI),
                    op=Alu.add,
                )

            def emit_relu(b, k):
                h110 = ht[b][k][0:HT, :]
                re = relu_eng.get((b, k), "act")
                if re == "act":
                    nc.scalar.activation(h110, h110, Act.Relu)
                elif re == "pool":
                    nc.gpsimd.tensor_tensor(
                        out=h110,
                        in0=h110,
                        in1=_view(zt[:, :], 0, [(0, WMAX), (1, L)]),
                        op=Alu.max,
                    )
                else:
                    nc.vector.tensor_scalar_max(h110, h110, 0.0)

            if emission == "stages":
                for b, k in tiles:
                    emit_odd(b, k)
                for b, k in tiles:
                    emit_even(b, k)
                for b, k in tiles:
                    emit_qadd(b, k, emit_qmul(b, k))
                for b, k in tiles:
                    emit_relu(b, k)
            else:
                for b, k in tiles:
                    emit_odd(b, k)
                    emit_even(b, k)
                    emit_qadd(b, k, emit_qmul(b, k))
                    emit_relu(b, k)

            # ---- phase 3: W2 matmul + log_softmax + store ----------------------
            # k-outer accumulation: chunk psum tiles stay resident across all
            # seven h k-tiles, so matmuls start as soon as each h tile's relu
            # lands instead of waiting for the whole batch.  Softmax reads the
            # psum directly (no logits staging copy).
            groups = [(0, 12), (12, 12), (24, NCH - 24)]
            for b in range(BL):
                fin = smp.tile([128, NCH * OUT], dt.float32, tag=f"fin{b}")
                ex = smp.tile([128, NCH * OUT], dt.float32, tag=f"ex{b}")
                ss = smp.tile([128, NCH], dt.float32, tag=f"ss{b}")
                lse = smp.tile([128, NCH], dt.float32, tag=f"lse{b}")
                pts = []
                for g in range(len(groups)):
                    pt = w2p.tile([128, 480], dt.float32, tag=f"w2ps_{b}_{g}")
                    pts.append(pt)
                # chunk-outer: each chunk's 7-matmul accumulation group runs
                # contiguously (interleaving start groups within a PSUM bank
                # corrupts accumulation), softmax per group as soon as its
                # chunks complete.
                for g, (c0, n) in enumerate(groups):
                    for j in range(n):
                        c = c0 + j
                        for k in range(NK):
                            kk = HT + 1 if k == NK - 1 else HT
                            nc.tensor.matmul(
                                pts[g][:, OUT * j : OUT * j + OUT],
                                lhsT=ht[b][k][0:kk, L * c : L * c + L],
                                rhs=w2t[k][0:kk, :],
                                start=(k == 0),
                                stop=(k == NK - 1),
                            )
                for g, (c0, n) in enumerate(groups):
                    nc.scalar.activation(
                        ex[:, OUT * c0 : OUT * (c0 + n)], pts[g][:, 0 : OUT * n],
                        Act.Exp,
                    )
                    nc.vector.tensor_reduce(
                        out=ss[:, c0 : c0 + n],
                        in_=_view(ex[:, :], OUT * c0, [(OUT, n), (1, OUT)]),
                        axis=mybir.AxisListType.X,
                        op=Alu.add,
                    )
                    nc.scalar.activation(
                        lse[:, c0 : c0 + n], ss[:, c0 : c0 + n], Act.Ln
                    )
                    nc.vector.tensor_tensor(
                        out=_view(fin[:, :], OUT * c0, [(1, OUT), (OUT, n)]),
                        in0=_view(pts[g][:, :], 0, [(1, OUT), (OUT, n)]),
                        in1=_view(lse[:, :], c0, [(0, OUT), (1, n)]),
                        op=Alu.subtract,
                    )
                    # store each group as soon as its log-softmax lands
                    nc.sync.dma_start(
                        out=out[b][:, OUT * c0 : OUT * (c0 + n)],
                        in_=fin[:, OUT * c0 : OUT * (c0 + n)],
                    )

    return nc


def _default_cfg():
    # engine balance chosen via the InstructionCostModel timeline: Pool takes
    # the odd-diagonal assembly adds; relu mostly on ACT
    # Engine balance validated on HW at ~121us/iteration: relu on ACT,
    # odd-diagonal assembly adds on Pool, restricted strided q-passes on DVE.
    # (A rebalance toward DVE-relu + full-range contiguous q-passes measured
    # 136us -- worse -- so this split is the empirical optimum found.)
    return {
        "tt_odd": {(b, k) for b in range(BL) for k in range(NK)},
        "tt_even": set(),
        "relu": {},
        "emission": "tiles",
        "tile_order": "bk",
    }


def _host_prep(hidden_states, pred_spans, token_num, mask, W1, b1, W2, b2):
    hs = _f32(hidden_states)
    pred = np.asarray(pred_spans)
    W1 = _f32(W1)
    b1 = _f32(b1)
    W2f = _f32(W2)
    b2 = _f32(b2)
    tn = int(token_num)

    vecs = hs[:, 1 : tn + 1, :]                     # [B, L, D]
    W1a, W1b, w1c = W1[:D], W1[D : 2 * D], W1[2 * D]

    # per-core packed, augmented, transposed activations
    in_maps = []
    w1a_aug = np.zeros((DAUG, H1), np.float32)
    w1a_aug[0:D] = W1a
    w1a_aug[D] = w1c
    w1a_aug[D + 1] = b1
    w1c_aug = np.zeros((DAUG, H1), np.float32)
    w1c_aug[0:D] = W1b
    w1c_aug[D + 2] = -w1c
    wa_np = _bf16(w1a_aug.reshape(ND, 128, H1))
    wc_np = _bf16(w1c_aug.reshape(ND, 128, H1))
    w2cat = np.zeros((H1 + 1, OUT), np.float32)
    w2cat[0:H1] = W2f
    w2cat[H1] = b2
    w2_np = _bf16(w2cat)
    w1cc_np = _f32(w1c.reshape(H1, 1))

    ii = np.arange(L)
    q_region_ok = True
    for c in range(NCORES):
        va = np.zeros((DAUG, 128 * BL), np.float32)
        qrow = np.zeros((BL, FDH), np.float32)
        for b in range(BL):
            gb = BL * c + b
            s, e = int(pred[gb, 0]), int(pred[gb, 1])
            va[0:D, 128 * b : 128 * b + L] = vecs[gb].T
            va[D, 128 * b : 128 * b + L] = (ii >= s).astype(np.float32)
            va[D + 1, 128 * b : 128 * b + L] = 1.0
            va[D + 2, 128 * b : 128 * b + L] = (ii > e).astype(np.float32)
            for w in range(WMAX):
                i = ii[: L - w]
                j = i + w
                contained = (i < s) & (j > e)
                qrow[b, w * L : w * L + L - w] = contained.astype(np.float32)
                if e - s == w and s < L - w:
                    qrow[b, w * L + s] += 1.0
        # device applies the q pass only on i < 98 of each diagonal
        if qrow.reshape(BL, WMAX, L)[:, :, 98:].any():
            q_region_ok = False
        in_maps.append(
            dict(
                vp=_bf16(va.reshape(ND, 128, 128 * BL)),
                wa=wa_np,
                wc=wc_np,
                w2c=w2_np,
                w1cc=w1cc_np,
                qr=_bf16(qrow),
                ones_d=_bf16(np.ones((1, FDH), np.float32)),
            )
        )
    return in_maps if q_region_ok else None


def _fast_path_ok(hidden_states, pred_spans, token_num, mask):
    hs = np.asarray(hidden_states)
    mask = np.asarray(mask)
    if hs.shape != (B, L + 1, D) or int(token_num) != L:
        return False
    if mask.shape != (L, L):
        return False
    vi, vj = np.nonzero(mask == 1)
    w = vj - vi
    if w.min() < 0 or w.max() != WMAX - 1:
        return False
    # every width must be the full run i in [0, L - w)
    want = sum(L - ww for ww in range(WMAX))
    if len(vi) != want:
        return False
    for ww in range(WMAX):
        sel = vi[w == ww]
        if len(sel) != L - ww or not np.array_equal(np.sort(sel), np.arange(L - ww)):
            return False
    return True


def _reference_numpy(hidden_states, pred_spans, token_num, mask, W1, b1, W2, b2):
    """Exact fallback (host only) for input shapes the device program
    doesn't cover; mirrors reference.py semantics."""
    hs = _f32(hidden_states)
    mask = np.asarray(mask)
    tn = int(token_num)
    vi, vj = np.nonzero(mask == 1)
    vecs = hs[:, 1 : tn + 1, :]
    n = vecs.shape[1]
    vic = np.clip(vi, 0, n - 1)
    vjc = np.clip(vj, 0, n - 1)
    xi = vecs[:, vic, :]
    xj = vecs[:, vjc, :]
    s = np.asarray(pred_spans)[:, 0:1]
    e = np.asarray(pred_spans)[:, 1:2]
    exact = (vi[None, :] == s) & (vj[None, :] == e)
    inside = (vi[None, :] >= s) & (vj[None, :] <= e) & (vi[None, :] <= vj[None, :])
    ind = np.where(exact, 2.0, np.where(inside, 1.0, 0.0)).astype(np.float32)
    W1 = _f32(W1)
    Dd = vecs.shape[2]
    h = xi @ W1[:Dd] + xj @ W1[Dd : 2 * Dd] + ind[..., None] * W1[2 * Dd] + _f32(b1)
    h = np.maximum(h, 0.0)
    logits = h @ _f32(W2) + _f32(b2)
    m = logits.max(axis=-1, keepdims=True)
    z = np.exp(logits - m)
    return (logits - m - np.log(z.sum(axis=-1, keepdims=True))).astype(np.float32)


def kernel(**inputs):
    hidden_states = inputs["hidden_states"]
    pred_spans = inputs["pred_spans"]
    token_num = inputs["token_num"]
    mask = inputs["span_available_indication_matrix"]
    W1, b1, W2, b2 = inputs["W1"], inputs["b1"], inputs["W2"], inputs["b2"]

    if not _fast_path_ok(hidden_states, pred_spans, token_num, mask):
        return _reference_numpy(
            hidden_states, pred_spans, token_num, mask, W1, b1, W2, b2
        )

    from concourse.bass_utils import run_bass_kernel_spmd

    key = "v3"
    if key not in _prog_cache:
        _prog_cache[key] = _build_program(_default_cfg())
    nc = _prog_cache[key]

    in_maps = _host_prep(
        hidden_states, pred_spans, token_num, mask, W1, b1, W2, b2
    )
    if in_maps is None:
        return _reference_numpy(
            hidden_states, pred_spans, token_num, mask, W1, b1, W2, b2
        )
    res = run_bass_kernel_spmd(nc, in_maps, list(range(NCORES)))
    kernel.last_results = res

    # gather + un-permute: device emits [BL, span-in-chunk(=i), chunk(=w), OUT]
    mask = np.asarray(mask)
    vi, vj = np.nonzero(mask == 1)
    perm = (vj - vi) * L + vi                      # row-major span -> diag slot
    out = np.empty((B, len(vi), OUT), np.float32)
    for c in range(NCORES):
        o = (
            res.results[c]["out"]
            .reshape(BL, L, NCH, OUT)
            .transpose(0, 2, 1, 3)
            .reshape(BL, FDH, OUT)
        )
        for b in range(BL):
            out[BL * c + b] = o[b][perm]
    return out

